# revision 1
# baseline (speedup 1.0000x reference)
"""Bidirectional LSTM (B=64, T=1024, D=512, H=768) on 8 trn2 NeuronCores.

Sharding: direction x batch. Cores 0-3: forward LSTM, batch quarters of 16.
Cores 4-7: backward LSTM (host flips x in time), batch quarters of 16.
All 8 cores run the identical SPMD program; direction lives in the data.

Per-core kernel:
  phase A: xz = x @ W + b  (big GEMM, fp32r moving operand, M=128 row tiles)
           staged to HBM as [T, 6, 16, 512] so step slices DMA at full width.
  phase B: 1024 unrolled steps. Per step, 6 gate-permuted 512-col subchunks:
           PSUM <- eye.T @ xz_t(sub)  (injects xz into the accumulation group)
           PSUM += hT(k).T @ U(k, sub) for k in 0..5   (fp32r, 1 cyc/row)
           ACT: sigmoid(i,f,o) / tanh(g) from PSUM -> gates SBUF
           DVE: c = f*c + i*tg ; ACT: tanh(c) ; DVE: h = o*tanh(c)
           PE: transpose h slice -> hT for next step.
Gate columns of W/U/b are pre-permuted on the host so each 512-col subchunk
is [i(128)|f(128)|o(128)|g(128)] for 128 consecutive H units.
"""

import os
import sys

import numpy as np

for _p in ("/opt/trn_rl_repo",):
    if _p not in sys.path and os.path.isdir(_p):
        sys.path.insert(0, _p)

B, T, D, H = 64, 1024, 512, 768
G4 = 4 * H          # 3072 gate columns
NCORES = 8
BLOC = B // 4       # 16: batch per core (4 cores per direction)
NSUB = 6            # 512-col gate subchunks
SUB = G4 // NSUB    # 512
HSUB = H // NSUB    # 128 H units per subchunk
KCH = H // 128      # 6 contraction chunks of the recurrent matmul
DCH = D // 128      # 4 contraction chunks of the projection matmul
TWIN = 128 // BLOC  # 8 time steps per 128-row projection tile


def _build_nc(t_steps=T, mm_dt="f32r"):
    import concourse.bass as bass
    import concourse.mybir as mybir
    import concourse.tile as tile
    from concourse.bacc import Bacc
    from concourse.bass import ts

    f32 = mybir.dt.float32
    f32r = mybir.dt.float32r
    MMD = {"f32r": mybir.dt.float32r, "bf16": mybir.dt.bfloat16,
           "f32": mybir.dt.float32}[mm_dt]
    SIG = mybir.ActivationFunctionType.Sigmoid
    TANH = mybir.ActivationFunctionType.Tanh

    nc = Bacc()
    x_d = nc.declare_dram_parameter("x", [BLOC, t_steps, D], f32, isOutput=False)
    w_d = nc.declare_dram_parameter("w", [D, G4], MMD, isOutput=False)
    u_d = nc.declare_dram_parameter("u", [H, G4], MMD, isOutput=False)
    b_d = nc.declare_dram_parameter("b", [G4], f32, isOutput=False)
    eye_d = nc.declare_dram_parameter("eye", [128, 128], f32, isOutput=False)
    sel_d = nc.declare_dram_parameter("sel", [96, 2, BLOC], MMD, isOutput=False)
    hs_d = nc.declare_dram_parameter("hs", [t_steps, BLOC, H], f32, isOutput=True)

    n_mtiles = t_steps * BLOC // 128

    with tile.TileContext(nc) as tc:
        with (
            tc.tile_pool(name="const", bufs=1) as const_pool,
            tc.tile_pool(name="wsb", bufs=1) as w_pool,
            tc.tile_pool(name="xzdram", bufs=1, space="DRAM") as xzd_pool,
        ):
            xz_d = xzd_pool.tile([t_steps * BLOC // 128, NSUB, 128, SUB], MMD)
            eye_sb = const_pool.tile([128, 128], f32)
            nc.sync.dma_start(eye_sb[:], eye_d[:])
            sel_sb = const_pool.tile([96, 2, BLOC], MMD)
            nc.sync.dma_start(sel_sb[:], sel_d[:])
            eye16_sb = const_pool.tile([BLOC, BLOC], MMD)
            nc.sync.dma_start(eye16_sb[:], sel_d[0:BLOC, 0, :])
            bias_bc = const_pool.tile([128, G4], mybir.dt.bfloat16)
            _bap = b_d[:]
            nc.gpsimd.dma_start(
                bias_bc[:],
                bass.AP(tensor=_bap.tensor, offset=_bap.offset,
                        ap=[[0, 128]] + list(_bap.ap)),
            )

            w_sb = [w_pool.tile([128, G4], MMD, name=f"wsb{k}", tag=f"w{k}")
                    for k in range(DCH)]
            for k in range(DCH):
                nc.sync.dma_start(w_sb[k][:], w_d[ts(k, 128), :])
            u_sb = [w_pool.tile([128, G4], MMD, name=f"usb{k}", tag=f"u{k}")
                    for k in range(KCH)]
            for k in range(KCH):
                nc.sync.dma_start(u_sb[k][:], u_d[ts(k, 128), :])

            # ---- phase A: xz = x @ W + b -> DRAM staging ----
            with (
                tc.tile_pool(name="kxm", bufs=2) as kxm_pool,
                tc.tile_pool(name="apsum", bufs=2, space="PSUM") as apsum_pool,
                tc.tile_pool(name="aout", bufs=2) as aout_pool,
                tc.tile_pool(name="xzin", bufs=2) as xz_pool,
                tc.tile_pool(name="zpsum", bufs=2, space="PSUM") as z_pool,
                tc.tile_pool(name="tpsum", bufs=2, space="PSUM") as t_pool,
                tc.tile_pool(name="gates", bufs=2) as gate_pool,
                tc.tile_pool(name="state", bufs=1) as state_pool,
                tc.tile_pool(name="ht", bufs=2) as ht_pool,
                tc.tile_pool(name="tmp", bufs=2) as tmp_pool,
                tc.tile_pool(name="outb", bufs=3) as out_pool,
            ):
                for m in range(n_mtiles):
                    # rows r = t'*BLOC + b (t-major within the M-tile)
                    xrows = kxm_pool.tile([128, D], f32, tag="xrows")
                    for tp in range(TWIN):
                        nc.sync.dma_start(
                            xrows[ts(tp, BLOC), :], x_d[:, m * TWIN + tp, :]
                        )
                    xt_ps = apsum_pool.tile([128, D], f32, tag="xtps")
                    for k in range(DCH):
                        nc.tensor.transpose(
                            xt_ps[:, ts(k, 128)], xrows[:, ts(k, 128)], eye_sb[:]
                        )
                    kxm = kxm_pool.tile([128, D], MMD, tag="kxm")
                    nc.vector.tensor_copy(kxm[:], xt_ps[:])
                    for n in range(NSUB):
                        ps = apsum_pool.tile([128, SUB], f32, tag="ps")
                        for k in range(DCH):
                            nc.tensor.matmul(
                                ps[:],
                                kxm[:, ts(k, 128)],
                                w_sb[k][:, ts(n, SUB)],
                                start=(k == 0), stop=(k == DCH - 1),
                            )
                        ev = aout_pool.tile([128, SUB], MMD)
                        nc.vector.tensor_add(ev[:], ps[:], bias_bc[:, ts(n, SUB)])
                        nc.sync.dma_start(xz_d[m, n], ev[:])

                # ---- phase B: the recurrence ----
                tc.strict_bb_all_engine_barrier()
                c_sb = state_pool.tile([BLOC, H], f32)
                nc.vector.memset(c_sb[:], 0.0)
                z0 = state_pool.tile([128, KCH * BLOC], f32)
                nc.vector.memset(z0[:], 0.0)
                hT = ht_pool.tile([128, KCH * BLOC], MMD)
                nc.vector.tensor_copy(hT[:], z0[:])

                for t in range(t_steps):
                    xz_t = [
                        xz_pool.tile([BLOC, SUB], MMD, name=f"xzt{t}_{n}",
                                     tag=f"xz{n}")
                        for n in range(NSUB)
                    ]
                    for n in range(NSUB):
                        nc.sync.dma_start(
                            xz_t[n][:],
                            xz_d[t // TWIN, n, ts(t % TWIN, BLOC), :],
                        )
                    h_t = out_pool.tile([BLOC, H], f32)
                    psT = t_pool.tile([128, KCH * BLOC], f32)
                    gates = gate_pool.tile([BLOC, 4, H], f32)
                    hT_next = ht_pool.tile([128, KCH * BLOC], MMD)

                    for n in range(NSUB):
                        zps = z_pool.tile([BLOC, SUB], f32)
                        nc.tensor.matmul(
                            zps[:], eye16_sb[:],
                            xz_t[n][:],
                            start=True, stop=False,
                        )
                        for k in range(KCH):
                            nc.tensor.matmul(
                                zps[:],
                                hT[:, ts(k, BLOC)],
                                u_sb[k][:, ts(n, SUB)],
                                start=False, stop=(k == KCH - 1),
                            )
                        # gates: [i|f|o] sigmoid, [g] tanh
                        nc.scalar.activation(
                            gates[:, 0:3, ts(n, HSUB)],
                            zps[:, 0:3 * HSUB].rearrange("p (g h) -> p g h", g=3),
                            SIG,
                        )
                        nc.scalar.activation(
                            gates[:, 3, ts(n, HSUB)], zps[:, 3 * HSUB:SUB], TANH,
                        )
                        i_g = gates[:, 0, ts(n, HSUB)]
                        f_g = gates[:, 1, ts(n, HSUB)]
                        o_g = gates[:, 2, ts(n, HSUB)]
                        tg_g = gates[:, 3, ts(n, HSUB)]
                        c_sl = c_sb[:, ts(n, HSUB)]
                        t1 = tmp_pool.tile([BLOC, HSUB], f32, tag="t1")
                        t2 = tmp_pool.tile([BLOC, HSUB], f32, tag="t2")
                        tcn = tmp_pool.tile([BLOC, HSUB], f32, tag="tc")
                        nc.vector.tensor_mul(t1[:], i_g, tg_g)
                        nc.vector.tensor_mul(t2[:], f_g, c_sl)
                        nc.vector.tensor_add(c_sl, t1[:], t2[:])
                        nc.scalar.activation(tcn[:], c_sl, TANH)
                        h_sl = h_t[:, ts(n, HSUB)]
                        nc.vector.tensor_mul(h_sl, o_g, tcn[:])
                        nc.tensor.transpose(
                            psT[:, ts(n, BLOC)], h_sl, eye_sb[:BLOC, :BLOC]
                        )

                    nc.vector.tensor_copy(hT_next[:], psT[:])
                    hT = hT_next
                    nc.sync.dma_start(hs_d[t], h_t[:])
    nc.finalize()
    return nc


_NC_CACHE = {}


def _get_nc(t_steps=T, mm_dt="f32r"):
    key = (t_steps, mm_dt)
    if key not in _NC_CACHE:
        _NC_CACHE[key] = _build_nc(t_steps, mm_dt)
    return _NC_CACHE[key]


def _gate_perm():
    """Permutation of the 3072 gate cols: subchunk n = [i|f|o|g] for
    H units [128n, 128n+128). Reference gate order in z is i,f,g,o."""
    p = np.empty(G4, dtype=np.int64)
    for n in range(NSUB):
        s = n * SUB
        hs = n * HSUB
        p[s:s + HSUB] = np.arange(hs, hs + HSUB)                # i
        p[s + HSUB:s + 2 * HSUB] = H + np.arange(hs, hs + HSUB)      # f
        p[s + 2 * HSUB:s + 3 * HSUB] = 3 * H + np.arange(hs, hs + HSUB)  # o
        p[s + 3 * HSUB:s + 4 * HSUB] = 2 * H + np.arange(hs, hs + HSUB)  # g
    return p


LAST_EXEC_NS = None


def kernel(x, Wf, Uf, bf, Wb, Ub, bb):
    global LAST_EXEC_NS
    from concourse.bass_utils import run_bass_kernel_spmd

    x = np.ascontiguousarray(np.asarray(x, dtype=np.float32))
    perm = _gate_perm()
    Wf_p = np.ascontiguousarray(np.asarray(Wf, np.float32)[:, perm])
    Uf_p = np.ascontiguousarray(np.asarray(Uf, np.float32)[:, perm])
    bf_p = np.ascontiguousarray(np.asarray(bf, np.float32)[perm])
    Wb_p = np.ascontiguousarray(np.asarray(Wb, np.float32)[:, perm])
    Ub_p = np.ascontiguousarray(np.asarray(Ub, np.float32)[:, perm])
    bb_p = np.ascontiguousarray(np.asarray(bb, np.float32)[perm])
    eye = np.eye(128, dtype=np.float32)
    sel = np.zeros((96, 2, BLOC), dtype=np.float32)
    for j in range(3):
        sel[32 * j:32 * j + BLOC, 0] = eye[:BLOC, :BLOC]
        sel[32 * j + BLOC:32 * j + 32, 1] = eye[:BLOC, :BLOC]

    in_maps = []
    for c in range(NCORES):
        fwd = c < 4
        bsl = slice((c % 4) * BLOC, (c % 4 + 1) * BLOC)
        xs = x[bsl] if fwd else x[bsl, ::-1]
        in_maps.append({
            "x": np.ascontiguousarray(xs),
            "w": Wf_p if fwd else Wb_p,
            "u": Uf_p if fwd else Ub_p,
            "b": bf_p if fwd else bb_p,
            "eye": eye,
            "sel": sel,
        })

    nc = _get_nc(T)
    trace = bool(int(os.environ.get("BASS_KERNEL_TRACE", "0")))
    res = run_bass_kernel_spmd(
        nc, in_maps, core_ids=list(range(NCORES)), trace=trace
    )
    LAST_EXEC_NS = getattr(res, "exec_time_ns", None)
    out = np.empty((B, T, 2 * H), dtype=np.float32)
    for c in range(NCORES):
        hs = res.results[c]["hs"]          # [T, BLOC, H]
        bsl = slice((c % 4) * BLOC, (c % 4 + 1) * BLOC)
        if c < 4:
            out[bsl, :, :H] = hs.transpose(1, 0, 2)
        else:
            out[bsl, :, H:] = hs.transpose(1, 0, 2)[:, ::-1]
    return out

